# revision 38
# baseline (speedup 1.0000x reference)
"""Trainium2 Bass kernel for the 2-layer GAT model (top-10 attention, 4 heads).

Strategy (8 NeuronCores, SPMD):
- Nodes sharded into 8 contiguous ranges of 6250 (dst ranges == GEMM shards).
- Per core, dst nodes are degree-sorted into 49 tiles of 128 with a common
  per-tile slot count D[t] (shared across cores: one NEFF serves all 8).
- Feature columns are HEAD-INTERLEAVED ((c,h) order, h innermost).
- xl = x @ W.T runs as a distributed GEMM over the shard (layer 1 reads
  host-pretransposed x, layer 2 PE-transposes), with a_s/a_d folded in as
  8 extra columns. Layer 1 replicates via one early AllGather; layer 2 via
  a chunk-major-layout AllGather streamed behind the fused edge1+gemm2
  phase. PSUM->SBUF copies run on the Activation engine; the attention
  weighting splits between DVE and Pool (gpsimd).
- Layer-1 node-table rows are 1280B f32 (256 xl | 4 a_s | pad): the final
  output is chaotically sensitive to layer-1 message precision (top-10
  mask flips in layer 2), so layer 1 stays f32. Layer-2 rows are 768B bf16
  (256 xl bf16 | 4 f32 a_s bit-packed | pad): quantizing layer 2 only
  costs ~4e-3 rel err. One AllGather per layer replicates each table.
- Edge phase per tile: dma_gather fetches fused rows (layer 1: 28-slot
  chunks; layer 2: whole tile) via SBUF-preloaded index tables; alpha = leaky_relu(a_s + a_d) in
  [128, D, H] layout; top-10 per (dst, head) via DVE max8 + match_replace
  + max8; ex = exp(alpha - max) * masks; weighting: layer 1 multiplies in
  f32 and accumulates with f32 identity matmuls, layer 2 multiplies in
  packed bf16 (DVE 2x_1p) and accumulates with bf16 identity matmuls. Layer 2 adds head-mean + 2-layer MLP per tile; results
  unpermuted on host.
"""
import numpy as np

N = 50000
E = 800000
F_IN = 256
H, C = 4, 64
HC = H * C
XC1 = 320                    # f32 cols per L1 table row (1280B): 256 xl | 4 as | pad
XC2 = 384                    # bf16 cols per L2 table row (768B): 256 xl | 8 as | pad
ACOL2 = 128                  # f32 col offset of a_s in the bitcast L2 row view
JC1 = 28                     # L1 gather chunk (slots per dma_gather)
K_TOP = 10
NEG_SLOPE = 0.2
N_CORES = 8
SH = N // N_CORES            # 6250 real rows per core
TILES = (SH + 127) // 128    # 49
ROWS = TILES * 128           # 6272 padded rows per core
NT = N_CORES * ROWS          # 50176 global table rows
BASE = 32768                 # signed-int16 gather base row
PADROW = NT - 1              # ghost row of last core (content = zeros)
HID, OUT_F = 128, 16

# AllGather chunking: tiles per chunk (uneven ok; cumulative row offsets)
CH_TILES = [25, 24]
CH_ROWS = [t * 128 for t in CH_TILES]
CH_START = np.repeat(np.cumsum([0] + CH_ROWS[:-1]), CH_ROWS)  # loc -> chunk start row
CH_BASE = np.cumsum([0] + CH_ROWS[:-1])                        # per-chunk shard offset
CHUNK_OF = np.repeat(np.arange(len(CH_TILES)), CH_ROWS)        # loc -> chunk id
CH_BASE = np.asarray(CH_BASE)
CH_START_V = np.asarray(np.cumsum([0] + CH_ROWS[:-1]))

# (c,h) head-interleaved column order: new index c*H+h <- old index h*C+c
PERM = np.empty(HC, np.int64)
for _h in range(H):
    for _c in range(C):
        PERM[_c * H + _h] = _h * C + _c


def _wrap_idx(vals: np.ndarray) -> np.ndarray:
    """int16 index list -> [128, ceil(len/16)] wrapped+replicated tile."""
    ni = len(vals)
    w = -(-ni // 16)
    arr = np.full(w * 16, PADROW - BASE, np.int16)
    arr[:ni] = vals
    return np.tile(arr.reshape(w, 16).T, (8, 1))


def _chunks(Dt):
    return [(j0, min(JC1, Dt - j0)) for j0 in range(0, Dt, JC1)]


def _prep(x, W1, att_s1, att_d1, W2, att_s2, att_d2, Wl1, Wl2, edge_index):
    """Host preprocessing: sharding, degree-sorted tiles, gather index tables."""
    src = np.asarray(edge_index[0], np.int64)
    dst = np.asarray(edge_index[1], np.int64)

    deg = np.bincount(dst, minlength=N)
    loc = np.empty(N, np.int64)
    node_of = np.full((N_CORES, ROWS), -1, np.int64)  # local row -> global node
    for c in range(N_CORES):
        nodes = np.arange(c * SH, (c + 1) * SH)
        order = np.argsort(-deg[nodes], kind="stable")
        loc[nodes[order]] = np.arange(SH)
        node_of[c, :SH] = nodes[order]
    _c = np.arange(N) // SH
    # both layers: chunk-major [chunk][core][row-in-chunk] table layout
    _k = CHUNK_OF[loc]
    _chrows = np.asarray(CH_ROWS)[_k]
    rowid1 = (CH_BASE[_k] * N_CORES + _c * _chrows + (loc - CH_START_V[_k]))
    rowid2 = rowid1

    degl = np.zeros((N_CORES, ROWS), np.int64)
    for c in range(N_CORES):
        degl[c, :SH] = deg[node_of[c, :SH]]
    tile_max = degl.reshape(N_CORES, TILES, 128).max(axis=(0, 2))
    D = np.maximum(8, ((tile_max + 3) // 4) * 4).astype(np.int64)
    assert D.max() <= 120, f"degree too high for this kernel: {D.max()}"

    e_loc = (dst // SH) * ROWS + loc[dst]
    order_e = np.argsort(e_loc, kind="stable")
    src_s = src[order_e]
    e_loc_s = e_loc[order_e]
    starts = np.searchsorted(e_loc_s, np.arange(N_CORES * ROWS))
    ends = np.searchsorted(e_loc_s, np.arange(N_CORES * ROWS) + 1)

    PAD16 = np.int16(PADROW - BASE)
    idx_full_parts = [[] for _ in range(N_CORES)]   # whole-tile (L2)
    idx_chunk_parts = [[] for _ in range(N_CORES)]  # JC1-slot chunks (L1)
    for c in range(N_CORES):
        for t in range(TILES):
            Dt = int(D[t])
            slot1 = np.full((128, Dt), PADROW, np.int64)
            slot2 = np.full((128, Dt), PADROW, np.int64)
            for d in range(128):
                r = c * ROWS + t * 128 + d
                s, e = starts[r], ends[r]
                if e > s:
                    slot1[d, : e - s] = rowid1[src_s[s:e]]
                    slot2[d, : e - s] = rowid2[src_s[s:e]]
            s16f = (slot2 - BASE).astype(np.int16)
            jm = s16f.T.reshape(-1)  # [D*128] slot-major
            idx_full_parts[c].append(_wrap_idx(np.concatenate([jm, [PAD16]])))
            s16c = (slot1 - BASE).astype(np.int16)
            for (j0, jc) in _chunks(Dt):
                part = s16c[:, j0 : j0 + jc].T.reshape(-1)
                idx_chunk_parts[c].append(_wrap_idx(np.concatenate([part, [PAD16]])))
    idx_full = np.stack([np.concatenate(p, axis=1) for p in idx_full_parts])
    idx_chunk = np.stack([np.concatenate(p, axis=1) for p in idx_chunk_parts])

    degf = np.zeros((N_CORES, 128, TILES), np.float32)
    for c in range(N_CORES):
        degf[c] = degl[c].reshape(TILES, 128).T.astype(np.float32)

    xT_shard = np.zeros((N_CORES, F_IN, ROWS), np.float32)
    for c in range(N_CORES):
        xT_shard[c, :, :SH] = np.asarray(x)[node_of[c, :SH]].T

    def att_fold(WT, att_s, att_d):
        Vs = np.stack([WT[:, h * C : (h + 1) * C] @ np.asarray(att_s)[0, h]
                       for h in range(H)], axis=1)
        Vd = np.stack([WT[:, h * C : (h + 1) * C] @ np.asarray(att_d)[0, h]
                       for h in range(H)], axis=1)
        return np.hstack([WT[:, PERM], Vs, Vd]).astype(np.float32)

    W1T = np.asarray(W1).T.astype(np.float32)
    W2T = np.asarray(W2).T.astype(np.float32)
    W1T_ext = att_fold(W1T, att_s1, att_d1)
    W2T_ext = att_fold(W2T[PERM, :], att_s2, att_d2)  # rows in (c,h) order too

    meta = dict(D=[int(d) for d in D])
    consts = dict(
        W1T_ext=W1T_ext, W2T_ext=W2T_ext,
        Wl1T=np.asarray(Wl1).T.astype(np.float32).copy(),
        Wl2T=np.asarray(Wl2).T.astype(np.float32).copy(),
    )
    per_core = dict(xT_shard=xT_shard, idx_full=idx_full, idx_chunk=idx_chunk,
                    degf=degf)
    return meta, consts, per_core, node_of


def build_gnn(meta, repeat=1):
    from concourse import bass, bacc, mybir
    import concourse.tile as tile
    from concourse.masks import make_identity

    D = meta["D"]
    WFULL = sum(-(-(128 * D[t] + 1) // 16) for t in range(TILES))
    WCHUNK = sum(-(-(128 * jc + 1) // 16)
                 for t in range(TILES) for (_, jc) in _chunks(D[t]))

    f32 = mybir.dt.float32
    f32r = mybir.dt.float32r
    bf16 = mybir.dt.bfloat16
    i16 = mybir.dt.int16
    nc = bacc.Bacc(None, target_bir_lowering=False, num_devices=N_CORES,
                   num_swdge_queues=4)

    # inputs
    xT_in = nc.dram_tensor("xT_shard", [F_IN, ROWS], f32, kind="ExternalInput")
    w1_in = nc.dram_tensor("W1T_ext", [F_IN, HC + 8], f32, kind="ExternalInput")
    w2_in = nc.dram_tensor("W2T_ext", [HC, HC + 8], f32, kind="ExternalInput")
    wl1_in = nc.dram_tensor("Wl1T", [C, HID], f32, kind="ExternalInput")
    wl2_in = nc.dram_tensor("Wl2T", [HID, OUT_F], f32, kind="ExternalInput")
    bl1_in = nc.dram_tensor("bl1_col", [HID, 1], f32, kind="ExternalInput")
    bl2_in = nc.dram_tensor("bl2_rep", [128, OUT_F], f32, kind="ExternalInput")
    b1_in = nc.dram_tensor("b1_rep", [128, HC], f32, kind="ExternalInput")
    b2_in = nc.dram_tensor("b2_rep", [128, C], f32, kind="ExternalInput")
    dg_in = nc.dram_tensor("degf", [128, TILES], f32, kind="ExternalInput")
    if_in = nc.dram_tensor("idx_full", [128, WFULL], i16, kind="ExternalInput")
    ic_in = nc.dram_tensor("idx_chunk", [128, WCHUNK], i16, kind="ExternalInput")

    out_dram = nc.dram_tensor("out", [ROWS, OUT_F], f32, kind="ExternalOutput")

    # internal DRAM: fused node-table rows
    xl1_sh = nc.dram_tensor("xl1_shard", [ROWS, XC1], f32)
    xl2_sh = nc.dram_tensor("xl2_shard", [ROWS, XC2], bf16)
    ad_lo = [nc.dram_tensor(f"ad{l}_local", [ROWS, 4], f32) for l in (1, 2)]
    xl1_fu = nc.dram_tensor("xl1_full", [NT, XC1], f32, addr_space="Shared")
    xl2_fu = nc.dram_tensor("xl2_full", [NT, XC2], bf16, addr_space="Shared")

    CPY = mybir.ActivationFunctionType.Copy
    MAXO = mybir.AluOpType.max
    EXP = mybir.ActivationFunctionType.Exp
    RELU = mybir.ActivationFunctionType.Relu
    ADD = mybir.AluOpType.add
    MUL = mybir.AluOpType.mult
    SUB = mybir.AluOpType.subtract
    GE = mybir.AluOpType.is_ge
    LT = mybir.AluOpType.is_lt

    with tile.TileContext(nc) as tc:
        with (
            tc.tile_pool(name="const", bufs=1) as cpool,
            tc.tile_pool(name="gemm", bufs=3) as gpool,
            tc.tile_pool(name="gpsum", bufs=2, space="PSUM") as gpsum,
            tc.tile_pool(name="edge1", bufs=2) as epool1,
            tc.tile_pool(name="edge2", bufs=2) as epool2,
            tc.tile_pool(name="small", bufs=4) as spool,
            tc.tile_pool(name="agg", bufs=2, space="PSUM") as apsum,
            tc.tile_pool(name="mlpp", bufs=1, space="PSUM") as mpsum,
        ):
            # ---- constants ----
            ident = cpool.tile([128, 128], f32)
            make_identity(nc, ident[:])
            identb = cpool.tile([128, 128], bf16)
            nc.vector.tensor_copy(identb[:], ident[:])
            iota_i = cpool.tile([128, 128], mybir.dt.int32)
            nc.gpsimd.iota(iota_i[:], pattern=[[1, 128]], base=0, channel_multiplier=0)
            iota_f = cpool.tile([128, 128], f32)
            nc.vector.tensor_copy(iota_f[:], iota_i[:])
            w1_sb = cpool.tile([128, 2, HC + 8], f32)
            nc.sync.dma_start(out=w1_sb[:, 0], in_=w1_in[0:128])
            nc.sync.dma_start(out=w1_sb[:, 1], in_=w1_in[128:256])
            w2_sb = cpool.tile([128, 2, HC + 8], f32)
            nc.sync.dma_start(out=w2_sb[:, 0], in_=w2_in[0:128])
            nc.sync.dma_start(out=w2_sb[:, 1], in_=w2_in[128:256])
            wl1_sb = cpool.tile([C, HID], f32)
            nc.sync.dma_start(out=wl1_sb[:], in_=wl1_in[:])
            wl2_sb = cpool.tile([HID, OUT_F], f32)
            nc.sync.dma_start(out=wl2_sb[:], in_=wl2_in[:])
            bl1_sb = cpool.tile([HID, 1], f32)
            nc.sync.dma_start(out=bl1_sb[:], in_=bl1_in[:])
            bl2_sb = cpool.tile([128, OUT_F], f32)
            nc.sync.dma_start(out=bl2_sb[:], in_=bl2_in[:])
            b1_sb = cpool.tile([128, HC], f32)
            nc.sync.dma_start(out=b1_sb[:], in_=b1_in[:])
            b2_sb = cpool.tile([128, C], f32)
            nc.sync.dma_start(out=b2_sb[:], in_=b2_in[:])
            deg_sb = cpool.tile([128, TILES], f32)
            nc.sync.dma_start(out=deg_sb[:], in_=dg_in[:])
            iall_c = cpool.tile([128, WCHUNK], i16)
            nc.sync.dma_start(out=iall_c[:], in_=ic_in[:])
            iall_f = cpool.tile([128, WFULL], i16)
            nc.sync.dma_start(out=iall_f[:], in_=if_in[:])

            def store_shard(l, t, ps):
                rows = slice(t * 128, (t + 1) * 128)
                oa = gpool.tile([128, 8], f32, tag="g_asd")
                nc.scalar.activation(oa[:], ps[:, HC : HC + 8], CPY)
                if l == 1:
                    og = gpool.tile([128, HC], f32, tag="g_out1")
                    nc.scalar.activation(og[:], ps[:, :HC], CPY)
                    nc.sync.dma_start(out=xl1_sh[rows, 0:HC], in_=og[:])
                    nc.sync.dma_start(out=xl1_sh[rows, HC : HC + 4], in_=oa[:, 0:4])
                else:
                    og = gpool.tile([128, HC], bf16, tag="g_out2")
                    nc.scalar.activation(og[:], ps[:, :HC], CPY)
                    nc.sync.dma_start(out=xl2_sh[rows, 0:HC], in_=og[:])
                    nc.sync.dma_start(
                        out=xl2_sh.ap().bitcast(f32)[rows, ACOL2 : ACOL2 + 4],
                        in_=oa[:, 0:4],
                    )
                nc.sync.dma_start(out=ad_lo[l - 1][rows], in_=oa[:, 4:8])

            def gemm1():
                for t in range(TILES):
                    rows = slice(t * 128, (t + 1) * 128)
                    xT = gpool.tile([128, 2, 128], f32, tag="g_T")
                    nc.sync.dma_start(out=xT[:, 0], in_=xT_in[0:128, rows])
                    nc.sync.dma_start(out=xT[:, 1], in_=xT_in[128:256, rows])
                    ps = gpsum.tile([128, HC + 8], f32, tag="g_mm")
                    nc.tensor.matmul(ps[:], xT[:, 0], w1_sb[:, 0], start=True, stop=False)
                    nc.tensor.matmul(ps[:], xT[:, 1], w1_sb[:, 1], start=False, stop=True)
                    store_shard(1, t, ps)
                    if t + 1 in CH_ENDS:
                        allgather_chunk(1, CH_ENDS.index(t + 1))

            def gemm2_tile(t, xt):
                xT = gpool.tile([128, 2, 128], f32, tag="g_T")
                for k in range(2):
                    pst = gpsum.tile([128, 128], f32, tag="g_tp")
                    nc.tensor.transpose(pst[:], xt[:, k * 128 : (k + 1) * 128], ident[:])
                    nc.scalar.activation(xT[:, k], pst[:], CPY)
                ps = gpsum.tile([128, HC + 8], f32, tag="g_mm")
                nc.tensor.matmul(ps[:], xT[:, 0], w2_sb[:, 0], start=True, stop=False)
                nc.tensor.matmul(ps[:], xT[:, 1], w2_sb[:, 1], start=False, stop=True)
                store_shard(2, t, ps)

            qrr = [0]
            CH_ENDS = list(np.cumsum(CH_TILES))

            def attention(t, Dt, alpha, l):
                """alpha [128,Dt,H] (already a_s+a_d) -> ex f32, inv."""
                lt1 = spool.tile([128, Dt, H], f32, tag="lt1")
                nc.scalar.activation(lt1[:], alpha[:], CPY, scale=NEG_SLOPE)
                nc.vector.tensor_tensor(out=alpha[:], in0=alpha[:], in1=lt1[:], op=MAXO)
                pm = spool.tile([128, Dt], f32, tag="pm")
                nc.vector.tensor_scalar(
                    out=pm[:], in0=iota_f[:, :Dt],
                    scalar1=deg_sb[:, t : t + 1], scalar2=None, op0=LT,
                )
                pb = spool.tile([128, Dt], f32, tag="pb")
                nc.vector.tensor_scalar(
                    out=pb[:], in0=pm[:], scalar1=1.0, scalar2=1e30,
                    op0=SUB, op1=MUL,
                )
                nc.vector.tensor_tensor(
                    out=alpha[:], in0=alpha[:],
                    in1=pb[:].unsqueeze(2).broadcast_to([128, Dt, H]), op=ADD,
                )
                m8s = spool.tile([128, H, 8], f32, tag="m8s")
                m8bs = spool.tile([128, H, 8], f32, tag="m8bs")
                if Dt > K_TOP:
                    for h in range(H):
                        nc.vector.max(out=m8s[:, h], in_=alpha[:, :, h])
                        wk = spool.tile([128, Dt], f32, tag="wk")
                        nc.vector.match_replace(
                            out=wk[:], in_to_replace=m8s[:, h],
                            in_values=alpha[:, :, h], imm_value=-3e30,
                        )
                        nc.vector.max(out=m8bs[:, h], in_=wk[:])
                else:
                    nc.vector.reduce_max(
                        out=m8s[:, :, 0], in_=alpha[:].transpose([0, 2, 1]),
                        axis=mybir.AxisListType.X,
                    )
                    nc.vector.memset(m8bs[:], -1e31)
                m_all = m8s[:, :, 0]   # [128, H] stride-8 view
                t10 = m8bs[:, :, 1]    # 10th-largest per (dst, head)
                ex = spool.tile([128, Dt, H], f32, tag="ex")
                nc.vector.tensor_tensor(
                    out=ex[:], in0=alpha[:],
                    in1=m_all.unsqueeze(1).broadcast_to([128, Dt, H]), op=SUB,
                )
                nc.scalar.activation(ex[:], ex[:], EXP)
                msk = spool.tile([128, Dt, H], f32, tag="msk")
                nc.vector.tensor_tensor(
                    out=msk[:], in0=alpha[:],
                    in1=t10.unsqueeze(1).broadcast_to([128, Dt, H]), op=GE,
                )
                nc.vector.tensor_tensor(out=ex[:], in0=ex[:], in1=msk[:], op=MUL)
                den = spool.tile([128, H], f32, tag="den")
                nc.vector.reduce_sum(
                    out=den[:], in_=ex[:].transpose([0, 2, 1]),
                    axis=mybir.AxisListType.X,
                )
                nc.vector.tensor_scalar_max(den[:], den[:], 1e-20)
                inv = spool.tile([128, H], f32, tag="inv")
                nc.vector.reciprocal(inv[:], den[:])
                if l == 2:
                    nc.vector.tensor_scalar_mul(inv[:], inv[:], 1.0 / H)
                return ex, inv

            def finish1(t, ps, inv):
                o = epool1.tile([128, HC], f32, tag="o1")
                nc.vector.tensor_tensor(
                    out=o[:].rearrange("p (c h) -> p c h", h=H),
                    in0=ps[:].rearrange("p (c h) -> p c h", h=H),
                    in1=inv[:].unsqueeze(1).broadcast_to([128, C, H]), op=MUL,
                )
                nc.vector.tensor_tensor(out=o[:], in0=o[:], in1=b1_sb[:], op=ADD)
                return o

            def finish2(t, ps, inv):
                rows = slice(t * 128, (t + 1) * 128)
                tmp = epool2.tile([128, HC], f32, tag="tmp2")
                nc.vector.tensor_tensor(
                    out=tmp[:].rearrange("p (c h) -> p c h", h=H),
                    in0=ps[:].rearrange("p (c h) -> p c h", h=H),
                    in1=inv[:].unsqueeze(1).broadcast_to([128, C, H]), op=MUL,
                )
                o2 = spool.tile([128, C], f32, tag="o2")
                nc.vector.reduce_sum(
                    out=o2[:], in_=tmp[:].rearrange("p (c h) -> p c h", h=H),
                    axis=mybir.AxisListType.X,
                )
                nc.vector.tensor_tensor(out=o2[:], in0=o2[:], in1=b2_sb[:], op=ADD)
                psT = mpsum.tile([C, 128], f32, tag="m_th")
                nc.tensor.transpose(psT[:], o2[:], ident[:])
                o2T = spool.tile([C, 128], f32, tag="o2T")
                nc.scalar.activation(o2T[:], psT[:], CPY)
                psh = mpsum.tile([HID, 128], f32, tag="m_th")
                nc.tensor.matmul(psh[:], wl1_sb[:], o2T[:], start=True, stop=True)
                rh = spool.tile([HID, 128], f32, tag="rh")
                nc.scalar.activation(rh[:], psh[:], RELU, bias=bl1_sb[:])
                pso = mpsum.tile([OUT_F, 128], f32, tag="m_of")
                nc.tensor.matmul(pso[:], wl2_sb[:], rh[:], start=True, stop=True)
                po = spool.tile([OUT_F, 128], f32, tag="po")
                nc.scalar.activation(po[:], pso[:], CPY)
                psf = mpsum.tile([128, OUT_F], f32, tag="m_of")
                nc.tensor.transpose(psf[:], po[:], ident[:OUT_F, :OUT_F])
                of = spool.tile([128, OUT_F], f32, tag="of")
                nc.vector.tensor_tensor(out=of[:], in0=psf[:], in1=bl2_sb[:], op=ADD)
                nc.sync.dma_start(out=out_dram[rows], in_=of[:])

            def load_ad(l):
                adr = ad_lo[l - 1].ap().rearrange("(t d) c -> d t c", d=128)
                ad_all = cpool.tile([128, TILES, 4], f32, tag=f"ad{l}")
                nc.sync.dma_start(out=ad_all[:], in_=adr[:])
                return ad_all

            def edge1():
                ad_all = load_ad(1)
                oc = 0
                for t in range(TILES):
                    Dt = D[t]
                    ch = _chunks(Dt)
                    xgs = []
                    alpha = spool.tile([128, Dt, H], f32, tag="alpha")
                    for (j0, jc) in ch:
                        nidx = 128 * jc + 1
                        wc = -(-nidx // 16)
                        ics = iall_c[:, oc : oc + wc]
                        oc += wc
                        xg = epool1.tile([128, jc + 1, XC1], f32, tag="xg1")
                        nc.gpsimd.dma_gather(
                            out_ap=xg[:], in_ap=xl1_fu[BASE:, :], idxs_ap=ics,
                            num_idxs=nidx, num_idxs_reg=nidx,
                            elem_size=XC1, single_packet=False, queue_num=qrr[0] % 4,
                        )
                        qrr[0] += 1
                        xgs.append((j0, jc, xg))
                        nc.vector.tensor_tensor(
                            out=alpha[:, j0 : j0 + jc],
                            in0=xg[:, :jc, HC : HC + 4],
                            in1=ad_all[:, t].unsqueeze(1).broadcast_to([128, jc, H]),
                            op=ADD,
                        )
                    ex, inv = attention(t, Dt, alpha, 1)
                    ps = apsum.tile([128, HC], f32, tag="agg")
                    for (j0, jc, xg) in xgs:
                        # split the f32 weighting ~2:1 between DVE and Pool
                        sp = (2 * jc + 2) // 3 if jc >= 6 else jc
                        nc.vector.tensor_tensor(
                            out=xg[:, :sp, 0:HC].rearrange("p j (c h) -> p j c h", h=H),
                            in0=xg[:, :sp, 0:HC].rearrange("p j (c h) -> p j c h", h=H),
                            in1=ex[:, j0 : j0 + sp].unsqueeze(2)
                                .broadcast_to([128, sp, C, H]),
                            op=MUL,
                        )
                        if sp < jc:
                            nc.gpsimd.tensor_tensor(
                                out=xg[:, sp:jc, 0:HC]
                                    .rearrange("p j (c h) -> p j c h", h=H),
                                in0=xg[:, sp:jc, 0:HC]
                                    .rearrange("p j (c h) -> p j c h", h=H),
                                in1=ex[:, j0 + sp : j0 + jc].unsqueeze(2)
                                    .broadcast_to([128, jc - sp, C, H]),
                                op=MUL,
                            )
                        for j in range(jc):
                            nc.tensor.matmul(
                                ps[:], ident[:], xg[:, j, 0:HC],
                                start=(j0 + j == 0), stop=(j0 + j == Dt - 1),
                            )
                    o1 = finish1(t, ps, inv)
                    gemm2_tile(t, o1[:])
                    if t + 1 in CH_ENDS:
                        allgather_chunk(2, CH_ENDS.index(t + 1))

            def edge2():
                ad_all = load_ad(2)
                of_ = 0
                for t in range(TILES):
                    Dt = D[t]
                    nidx = 128 * Dt + 1
                    wf = -(-nidx // 16)
                    ifs = iall_f[:, of_ : of_ + wf]
                    of_ += wf
                    xg = epool2.tile([128, Dt + 1, XC2], bf16, tag="xg2")
                    nc.gpsimd.dma_gather(
                        out_ap=xg[:], in_ap=xl2_fu[BASE:, :], idxs_ap=ifs,
                        num_idxs=nidx, num_idxs_reg=nidx,
                        elem_size=XC2, single_packet=False, queue_num=qrr[0] % 4,
                    )
                    qrr[0] += 1
                    xgf = xg[:].bitcast(f32)
                    alpha = spool.tile([128, Dt, H], f32, tag="alpha")
                    nc.vector.tensor_tensor(
                        out=alpha[:],
                        in0=xgf[:, :Dt, ACOL2 : ACOL2 + 4],
                        in1=ad_all[:, t].unsqueeze(1).broadcast_to([128, Dt, H]),
                        op=ADD,
                    )
                    ex, inv = attention(t, Dt, alpha, 2)
                    exb = spool.tile([128, Dt, H], bf16, tag="exb")
                    nc.scalar.activation(exb[:], ex[:], CPY)
                    sp2 = Dt - max(2, Dt // 5) if Dt >= 10 else Dt
                    nc.vector.tensor_tensor(
                        out=xg[:, :sp2, 0:HC].rearrange("p j (c h) -> p j c h", h=H),
                        in0=xg[:, :sp2, 0:HC].rearrange("p j (c h) -> p j c h", h=H),
                        in1=exb[:, :sp2].unsqueeze(2).broadcast_to([128, sp2, C, H]),
                        op=MUL,
                    )
                    if sp2 < Dt:
                        nc.gpsimd.tensor_tensor(
                            out=xg[:, sp2:Dt, 0:HC]
                                .rearrange("p j (c h) -> p j c h", h=H),
                            in0=xg[:, sp2:Dt, 0:HC]
                                .rearrange("p j (c h) -> p j c h", h=H),
                            in1=exb[:, sp2:Dt].unsqueeze(2)
                                .broadcast_to([128, Dt - sp2, C, H]),
                            op=MUL,
                        )
                    ps = apsum.tile([128, HC], f32, tag="agg")
                    for j in range(Dt):
                        nc.tensor.matmul(
                            ps[:], identb[:], xg[:, j, 0:HC],
                            start=(j == 0), stop=(j == Dt - 1),
                        )
                    finish2(t, ps, inv)

            def allgather_chunk(l, k):
                # chunk-major table layout: chunk k occupies contiguous rows
                sh, fu = (xl1_sh, xl1_fu) if l == 1 else (xl2_sh, xl2_fu)
                lo = int(CH_START_V[k])
                hi = lo + CH_ROWS[k]
                flo = int(CH_BASE[k]) * N_CORES
                nc.gpsimd.collective_compute(
                    "AllGather", mybir.AluOpType.bypass,
                    replica_groups=[list(range(N_CORES))],
                    ins=[sh[lo:hi].opt()],
                    outs=[fu[flo : flo + CH_ROWS[k] * N_CORES].opt()],
                )

            for _rep in range(repeat):
                gemm1()
                edge1()
                edge2()

    nc.compile()
    return nc


def _make_in_maps(consts, per_core, b1, b2, bl1, bl2):
    b1 = np.asarray(b1, np.float32)[PERM]
    b2 = np.asarray(b2, np.float32)
    bl1 = np.asarray(bl1, np.float32)
    bl2 = np.asarray(bl2, np.float32)
    shared = dict(
        W1T_ext=consts["W1T_ext"], W2T_ext=consts["W2T_ext"],
        Wl1T=consts["Wl1T"], Wl2T=consts["Wl2T"],
        bl1_col=np.ascontiguousarray(bl1[:, None]),
        bl2_rep=np.tile(bl2[None, :], (128, 1)),
        b1_rep=np.tile(b1[None, :], (128, 1)),
        b2_rep=np.tile(b2[None, :], (128, 1)),
    )
    return [
        dict(
            shared,
            xT_shard=np.ascontiguousarray(per_core["xT_shard"][c]),
            idx_full=np.ascontiguousarray(per_core["idx_full"][c]),
            idx_chunk=np.ascontiguousarray(per_core["idx_chunk"][c]),
            degf=np.ascontiguousarray(per_core["degf"][c]),
        )
        for c in range(N_CORES)
    ]


def _assemble(results, node_of):
    out = np.empty((N, OUT_F), np.float32)
    for c in range(N_CORES):
        out[node_of[c, :SH]] = results[c]["out"][:SH]
    return out


def kernel(x, W1, att_s1, att_d1, b1, W2, att_s2, att_d2, b2,
           Wl1, bl1, Wl2, bl2, edge_index):
    from concourse.bass_utils import run_bass_kernel_spmd

    meta, consts, per_core, node_of = _prep(
        x, W1, att_s1, att_d1, W2, att_s2, att_d2, Wl1, Wl2, edge_index
    )
    nc = build_gnn(meta)
    in_maps = _make_in_maps(consts, per_core, b1, b2, bl1, bl2)
    res = run_bass_kernel_spmd(nc, in_maps, core_ids=list(range(N_CORES)))
    return _assemble(res.results, node_of)
